# revision 1
# baseline (speedup 1.0000x reference)
"""NT-Xent loss kernel for Trainium2, 8-core SPMD.

Math: with p = cat(z_i, z_j) [8192, 64], pn = p / max(||p||, 1e-8),
sim = 2 * pn @ pn.T (TEMP=0.5), the reference's gather-based losses reduce to
  loss1 = mean_r( log(sum_{c != r} exp(sim[r,c])) - pos_r )
  loss2 = mean_r( log(exp(pos_r) + sum_{c != t_r} exp(probs[r,c])) - pos_r )
where pos_r = sim[r, (r+N) % 2N].  sim entries lie in [-2, 2], so the exp
never overflows and no max-shift pass is needed.  The huge neg_idx input is a
fixed structured mask (drop self + positive) and never needs to be read.

Sharding: row-parallel.  Each of the 8 cores gets 1024 rows of the sim matrix,
computes sum_c exp(2 * pn_shard @ pn.T) against the full all-rows pn (computed
redundantly on every core from the full p), plus its rows' pos/diag terms and
the probs part, and emits two partial sums.  Host adds the 8 partials.
"""

import numpy as np

import concourse.bass as bass
import concourse.bacc as bacc
import concourse.tile as tile
from concourse import mybir
from concourse.masks import make_identity
from concourse.bass_utils import run_bass_kernel_spmd

N = 4096
D = 64
M = 2 * N            # 8192 rows of sim
NCORES = 8
R = M // NCORES      # 1024 rows per core
NT = M // 128        # 64 row-tiles of the full p
NS = R // 128        # 8 row-tiles of a shard
NCLS = 10
INV_TEMP = 2.0       # 1 / 0.5
F32 = mybir.dt.float32
BF16 = mybir.dt.bfloat16

# bf16 matmul for the sim slab: 4x PE throughput, 2x moving-dim. pos/diag
# stay fp32 (computed on DVE), and per-row errors average out over 8192 rows.
import os
USE_BF16_MM = os.environ.get("NTX_BF16", "1") == "1"
USE_GPSIMD = os.environ.get("NTX_GPS", "0") == "1"
N_ACC_ENV = int(os.environ.get("NTX_NACC", "28"))
NEWTON_ITERS = int(os.environ.get("NTX_NEWT", "2"))
BENCH_REPS = int(os.environ.get("NTX_REPS", "0"))  # 0 = no loop

AF = mybir.ActivationFunctionType
ALU = mybir.AluOpType


def _emit_rsqrt(nc, pool, n2, nchunk, eng=None):
    """inv = 1/max(sqrt(n2), 1e-8), entirely on DVE: quake-style magic
    constant seed + 3 Newton steps (ACT stays exclusively on Exp/Ln, so the
    activation table never thrashes).  Newton converges the seed's 3.4% max
    error to below fp32 rounding."""
    if eng is None:
        eng = nc.vector
    I32 = mybir.dt.int32
    inv = pool.tile([128, nchunk], F32, tag="rs_inv")
    # seed: bits = 0x5f3759df - (bits(n2) >> 1)   (shift and arith must be
    # separate instructions -- walrus rejects mixed-class op0/op1)
    eng.tensor_scalar(inv.bitcast(I32), n2.bitcast(I32), 1, None,
                      ALU.arith_shift_right)
    eng.tensor_scalar(inv.bitcast(I32), inv.bitcast(I32), -1, 0x5F3759DF,
                      ALU.mult, ALU.add)
    t2 = pool.tile([128, nchunk], F32, tag="rs_t2")
    for _ in range(NEWTON_ITERS):
        # y' = y * (1.5 - 0.5 * n2 * y^2)
        eng.tensor_mul(t2, inv, inv)
        eng.tensor_mul(t2, t2, n2)
        eng.tensor_scalar(t2, t2, -0.5, 1.5, ALU.mult, ALU.add)
        eng.tensor_mul(inv, inv, t2)
    eng.tensor_scalar_min(inv, inv, 1e8)
    return inv


def _emit_normalize(nc, pool, raw, ntiles, tag, eng=None):
    """raw: [128, ntiles, 64] -> pn (same shape), rows normalized.

    The row scale is applied per 64-wide chunk with tensor_scalar_mul and a
    per-partition scalar AP (free-dim-broadcast APs with step 0 silently
    corrupt on HW, and tensor_tensor_reduce crashes the device).
    """
    if eng is None:
        eng = nc.gpsimd if USE_GPSIMD else nc.vector
    flat = raw.rearrange("p n d -> p (n d)")
    sq = pool.tile([128, ntiles * D], F32, tag=f"{tag}_sq")
    n2 = pool.tile([128, ntiles], F32, tag=f"{tag}_n2")
    eng.tensor_mul(sq, flat, flat)
    nc.vector.tensor_reduce(
        n2, sq.rearrange("p (n d) -> p n d", d=D), axis=mybir.AxisListType.X,
        op=ALU.add)
    inv = _emit_rsqrt(nc, pool, n2, ntiles, eng=eng)
    pn = pool.tile([128, ntiles, D], F32, tag=f"{tag}_pn")
    for n in range(ntiles):
        eng.tensor_scalar_mul(pn[:, n, :], raw[:, n, :],
                              inv[:, n:n + 1])
    return pn


def _emit_normalize_act(nc, pool, raw, ntiles, tag, out_dtype=None):
    """Prologue-only variant: squares and scale-muls run on the scalar
    engine (idle before the exp stream starts; Square/Copy live in every ACT
    table set so no swap), reduce+rsqrt on DVE.  Must not be used once the
    exp stream is running -- ACT queue order would stall it."""
    flat = raw.rearrange("p n d -> p (n d)")
    sq = pool.tile([128, ntiles * D], F32, tag=f"{tag}_sq")
    n2 = pool.tile([128, ntiles], F32, tag=f"{tag}_n2")
    nc.scalar.square(sq, flat)
    nc.vector.tensor_reduce(
        n2, sq.rearrange("p (n d) -> p n d", d=D), axis=mybir.AxisListType.X,
        op=ALU.add)
    inv = _emit_rsqrt(nc, pool, n2, ntiles)
    dt = out_dtype or F32
    pn = pool.tile([128, ntiles, D], dt, tag=f"{tag}_pn")
    for n in range(ntiles):
        nc.scalar.mul(pn[:, n, :], raw[:, n, :], inv[:, n:n + 1])
    return pn


def build_program():
    nc = bacc.Bacc("TRN2", target_bir_lowering=False, debug=False,
                   num_devices=NCORES)

    p_d = nc.dram_tensor("p", [M, D], F32, kind="ExternalInput").ap()
    psq_d = nc.dram_tensor("psq", [2 * R, D], F32,
                           kind="ExternalInput").ap()
    probs_d = nc.dram_tensor("probs", [R, NCLS], F32, kind="ExternalInput").ap()
    iota_d = nc.dram_tensor("iotah", [128, NCLS], F32,
                            kind="ExternalInput").ap()
    tgtr_d = nc.dram_tensor("tgtrep", [128, NS, NCLS], F32,
                            kind="ExternalInput").ap()
    out_d = nc.dram_tensor("out", [1, 2], F32, kind="ExternalOutput").ap()

    MMDT = BF16 if USE_BF16_MM else F32
    G = 4                 # stage-A column groups of the full p
    NTG = NT // G         # 16 row-chunks per group
    JJ = 8                # col groups of 1024 in the main loop
    # Bresenham split of the 64 row-sum reductions between ACT accum_out and
    # DVE tensor_reduce, to balance the two engines.
    N_ACC = N_ACC_ENV

    with tile.TileContext(nc) as tc:
        import contextlib
        with contextlib.ExitStack() as ctx:
            if BENCH_REPS > 1:
                ctx.enter_context(tc.For_i(0, BENCH_REPS, 1))
            consts = ctx.enter_context(tc.tile_pool(name="consts", bufs=1))
            big = ctx.enter_context(tc.tile_pool(name="big", bufs=1))
            work = ctx.enter_context(tc.tile_pool(name="work", bufs=2))
            grp = ctx.enter_context(tc.tile_pool(name="grp", bufs=2))
            tp = ctx.enter_context(
                tc.tile_pool(name="tp", bufs=3, space="PSUM"))
            mm = ctx.enter_context(
                tc.tile_pool(name="mm", bufs=2, space="PSUM"))
            po = ctx.enter_context(
                tc.tile_pool(name="po", bufs=1, space="PSUM"))
            esc = ctx.enter_context(tc.tile_pool(name="esc", bufs=6))

            identity = consts.tile([128, 128], MMDT)
            make_identity(nc, identity)
            iota10 = consts.tile([128, NCLS], F32)
            nc.sync.dma_start(out=iota10, in_=iota_d)
            ones = consts.tile([128, 1], F32)
            nc.vector.memset(ones, 1.0)

            eng = nc.gpsimd if USE_GPSIMD else nc.vector
            GROUPS = [16, 16, 16, 16]         # chunks per group, sum = NT
            goff = [0, 16, 32, 48]
            pnT = big.tile([64, M], MMDT)

            # prologue DMAs first so both chains can start immediately
            rawsq = big.tile([128, 2 * NS, D], F32)
            psq_r = psq_d.rearrange("(n p) d -> p n d", p=128)
            nc.sync.dma_start(out=rawsq[:, 0:NS, :], in_=psq_r[:, 0:NS, :])
            g0_raw = grp.tile([128, GROUPS[0], D], F32, tag="rawg",
                              padded_shape=[128, max(GROUPS), D])
            nc.sync.dma_start(
                out=g0_raw,
                in_=p_d.rearrange("(n p) d -> p n d", p=128)[
                    :, 0:GROUPS[0], :])
            nc.sync.dma_start(out=rawsq[:, NS:2 * NS, :],
                              in_=psq_r[:, NS:2 * NS, :])

            # Fine-grained prologue: compute the two rsqrt chains up front,
            # then interleave per-4-chunk normalize+transpose+copy so the
            # first psT / pnT columns (and with them the exp stream) are
            # ready as early as possible.  All on DVE: any op queued on ACT
            # ahead of the exps would head-of-line-block the stream.
            # ps half only (critical path to psT); pp half is tail-only
            # and is normalized after the stream has started.
            sflat = rawsq[:, 0:NS, :].rearrange("p n d -> p (n d)")
            s_sq = big.tile([128, NS * D], F32)
            s_n2 = big.tile([128, NS], F32)
            nc.vector.tensor_mul(s_sq, sflat, sflat)
            nc.vector.tensor_reduce(
                s_n2, s_sq.rearrange("p (n d) -> p n d", d=D),
                axis=mybir.AxisListType.X, op=ALU.add)
            s_inv = _emit_rsqrt(nc, big, s_n2, NS)

            gflat = g0_raw.rearrange("p n d -> p (n d)")
            g0_sq = grp.tile([128, GROUPS[0] * D], F32, tag="sqg",
                             padded_shape=[128, max(GROUPS) * D])
            g0_n2 = grp.tile([128, GROUPS[0]], F32, tag="n2g",
                             padded_shape=[128, max(GROUPS)])
            nc.vector.tensor_mul(g0_sq, gflat, gflat)
            nc.vector.tensor_reduce(
                g0_n2, g0_sq.rearrange("p (n d) -> p n d", d=D),
                axis=mybir.AxisListType.X, op=ALU.add)
            g0_inv = _emit_rsqrt(nc, grp, g0_n2, GROUPS[0], eng=eng)

            pnsq = big.tile([128, 2 * NS, D], F32)
            pns = pnsq[:, 0:NS, :]
            pnp = pnsq[:, NS:2 * NS, :]
            pnsb = pnsq if not USE_BF16_MM else big.tile([128, NS, D], BF16)
            psT = big.tile([64, R], MMDT)
            g0_pn = grp.tile([128, GROUPS[0], D], MMDT, tag="png",
                             padded_shape=[128, max(GROUPS), D])

            # shard chunks 0-3 -> psT[:, 0:512] first, then g0 chunks 0-7
            # -> pnT[:, 0:1024], then the remaining chunks of each.
            def shard_quad(q4):
                for n in range(4 * q4, 4 * q4 + 4):
                    nc.vector.tensor_scalar_mul(pnsq[:, n, :],
                                                rawsq[:, n, :],
                                                s_inv[:, n:n + 1])
                if USE_BF16_MM:
                    nc.vector.tensor_copy(
                        pnsb[:, 4 * q4:4 * q4 + 4, :].rearrange(
                            "p n d -> p (n d)"),
                        pnsq[:, 4 * q4:4 * q4 + 4, :].rearrange(
                            "p n d -> p (n d)"))
                tpp = tp.tile([64, 512], MMDT, tag="tp")
                for q in range(4):
                    nn = 4 * q4 + q
                    nc.tensor.transpose(
                        tpp[:, q * 128:(q + 1) * 128], pnsb[:, nn, :],
                        identity)
                nc.vector.tensor_copy(psT[:, q4 * 512:(q4 + 1) * 512], tpp)

            def g0_quad(q4):
                for n in range(4 * q4, 4 * q4 + 4):
                    nc.vector.tensor_scalar_mul(g0_pn[:, n, :],
                                                g0_raw[:, n, :],
                                                g0_inv[:, n:n + 1])
                tpp = tp.tile([64, 512], MMDT, tag="tp")
                for q in range(4):
                    nn = 4 * q4 + q
                    nc.tensor.transpose(
                        tpp[:, q * 128:(q + 1) * 128], g0_pn[:, nn, :],
                        identity)
                nc.vector.tensor_copy(pnT[:, q4 * 512:(q4 + 1) * 512], tpp)

            shard_quad(0)
            g0_quad(0)
            g0_quad(1)
            shard_quad(1)
            g0_quad(2)
            g0_quad(3)

            # pp half chain (tail-only data, after the stream is rolling)
            pflat = rawsq[:, NS:2 * NS, :].rearrange("p n d -> p (n d)")
            p_sq = big.tile([128, NS * D], F32)
            p_n2 = big.tile([128, NS], F32)
            nc.vector.tensor_mul(p_sq, pflat, pflat)
            nc.vector.tensor_reduce(
                p_n2, p_sq.rearrange("p (n d) -> p n d", d=D),
                axis=mybir.AxisListType.X, op=ALU.add)
            p_inv = _emit_rsqrt(nc, big, p_n2, NS)
            for n in range(NS):
                nc.vector.tensor_scalar_mul(pnsq[:, NS + n, :],
                                            rawsq[:, NS + n, :],
                                            p_inv[:, n:n + 1])

            probs_t = big.tile([128, NS, NCLS], F32)
            nc.sync.dma_start(
                out=probs_t, in_=probs_d.rearrange("(n p) c -> p n c", p=128))
            tgtr_t = big.tile([128, NS, NCLS], F32)
            nc.sync.dma_start(out=tgtr_t, in_=tgtr_d)
            eprobs = big.tile([128, NS, NCLS], F32)
            nc.scalar.activation(
                eprobs.rearrange("p n c -> p (n c)"),
                probs_t.rearrange("p n c -> p (n c)"), AF.Exp)

            # ---- full p, remaining pipelined column groups ----
            scols = big.tile([128, NS * JJ], F32)
            for g, ntg in enumerate(GROUPS):
                if g == 0:
                    png = None  # prologue already produced pnT cols
                else:
                    rawg = grp.tile([128, ntg, D], F32, tag="rawg",
                                    padded_shape=[128, max(GROUPS), D])
                    nc.sync.dma_start(
                        out=rawg,
                        in_=p_d.rearrange("(n p) d -> p n d", p=128)[
                            :, goff[g]:goff[g] + ntg, :])
                    flat = rawg.rearrange("p n d -> p (n d)")
                    sqg = grp.tile([128, ntg * D], F32, tag="sqg",
                                   padded_shape=[128, max(GROUPS) * D])
                    n2g = grp.tile([128, ntg], F32, tag="n2g",
                                   padded_shape=[128, max(GROUPS)])
                    eng.tensor_mul(sqg, flat, flat)
                    nc.vector.tensor_reduce(
                        n2g, sqg.rearrange("p (n d) -> p n d", d=D),
                        axis=mybir.AxisListType.X, op=ALU.add)
                    invg = _emit_rsqrt(nc, grp, n2g, ntg,
                                       eng=eng)
                    png = grp.tile([128, ntg, D], MMDT, tag="png",
                                   padded_shape=[128, max(GROUPS), D])
                    for n in range(ntg):
                        eng.tensor_scalar_mul(png[:, n, :], rawg[:, n, :],
                                              invg[:, n:n + 1])
                if g != 0:
                    for t4 in range(ntg // 4):
                        tpp = tp.tile([64, 512], MMDT, tag="tp")
                        for q in range(4):
                            nn = 4 * t4 + q
                            nc.tensor.transpose(
                                tpp[:, q * 128:(q + 1) * 128], png[:, nn, :],
                                identity)
                        col = (goff[g] + t4 * 4) * 128
                        nc.vector.tensor_copy(pnT[:, col:col + 512], tpp)

                # main loop for this group's columns
                jlo = goff[g] * 128 // 1024
                jhi = (goff[g] + ntg) * 128 // 1024
                for jj in range(jlo, jhi):
                    c0 = jj * 1024
                    for n in range(NS):
                        idx = n * JJ + jj
                        pst = mm.tile([128, 1024], F32, tag="mm")
                        lhsT = psT[:, n * 128:(n + 1) * 128]
                        nc.tensor.matmul(pst[:, 0:512], lhsT,
                                         pnT[:, c0:c0 + 512],
                                         start=True, stop=True)
                        nc.tensor.matmul(pst[:, 512:1024], lhsT,
                                         pnT[:, c0 + 512:c0 + 1024],
                                         start=True, stop=True)
                        et = esc.tile([128, 1024], F32, tag="esc")
                        if (idx * N_ACC) % (NS * JJ) < N_ACC:
                            nc.scalar.activation(
                                et, pst, AF.Exp, scale=INV_TEMP,
                                accum_out=scols[:, idx:idx + 1])
                        else:
                            nc.scalar.activation(et, pst, AF.Exp,
                                                 scale=INV_TEMP)
                            nc.vector.tensor_reduce(
                                scols[:, idx:idx + 1], et,
                                axis=mybir.AxisListType.X, op=ALU.add)

            sum10 = big.tile([128, NS], F32)
            nc.vector.tensor_reduce(sum10, eprobs, axis=mybir.AxisListType.X,
                                    op=ALU.add)
            own = big.tile([128, NS], F32)
            for n in range(NS):
                mask = work.tile([128, NCLS], F32, tag="mask")
                nc.vector.tensor_tensor(mask, iota10, tgtr_t[:, n, :],
                                        ALU.is_equal)
                nc.vector.tensor_mul(mask, mask, eprobs[:, n, :])
                nc.vector.tensor_reduce(own[:, n:n + 1], mask,
                                        axis=mybir.AxisListType.X, op=ALU.add)

            # pos_r and diag_r row-dots in fp32 (raw, without *2 temp
            # scale) — emitted late so their DVE/ACT ops cannot stall the
            # main exp stream (engine queues respect program order).
            diag_raw = big.tile([128, NS], F32)
            pos_raw = big.tile([128, NS], F32)
            dq = work.tile([128, NS, D], F32, tag="rowdot", bufs=2)
            nc.vector.tensor_mul(dq, pns, pns)
            nc.vector.tensor_reduce(diag_raw, dq, axis=mybir.AxisListType.X,
                                    op=ALU.add)
            pq = work.tile([128, NS, D], F32, tag="rowdot", bufs=2)
            nc.vector.tensor_mul(pq, pns, pnp)
            nc.vector.tensor_reduce(pos_raw, pq, axis=mybir.AxisListType.X,
                                    op=ALU.add)
            ediag = big.tile([128, NS], F32)
            nc.scalar.activation(ediag, diag_raw, AF.Exp, scale=INV_TEMP)
            epos = big.tile([128, NS], F32)
            nc.scalar.activation(epos, pos_raw, AF.Exp, scale=INV_TEMP)
            pos2 = big.tile([128, NS], F32)
            nc.vector.tensor_scalar_mul(pos2, pos_raw, INV_TEMP)

            # ---- loss tails ----
            stot = big.tile([128, NS], F32)
            nc.vector.tensor_reduce(
                stot, scols.rearrange("p (n j) -> p n j", j=JJ),
                axis=mybir.AxisListType.X, op=ALU.add)
            s1 = big.tile([128, NS], F32)
            nc.vector.tensor_sub(s1, stot, ediag)
            lse1 = big.tile([128, NS], F32)
            nc.scalar.activation(lse1, s1, AF.Ln)
            c1 = big.tile([128, NS], F32)
            nc.vector.tensor_sub(c1, lse1, pos2)
            v12 = big.tile([128, 2], F32)
            nc.vector.tensor_reduce(v12[:, 0:1], c1,
                                    axis=mybir.AxisListType.X, op=ALU.add)

            s2 = big.tile([128, NS], F32)
            nc.vector.tensor_sub(s2, sum10, own)
            nc.vector.tensor_add(s2, s2, epos)
            # false data-dep on stot so the scheduler cannot hoist the Ln
            # into the exp stream (each hoist costs 2 ACT table swaps)
            nc.vector.scalar_tensor_tensor(
                out=s2, in0=stot, scalar=0.0, in1=s2,
                op0=ALU.mult, op1=ALU.add)
            lse2 = big.tile([128, NS], F32)
            nc.scalar.activation(lse2, s2, AF.Ln)
            c2 = big.tile([128, NS], F32)
            nc.vector.tensor_sub(c2, lse2, pos2)
            nc.vector.tensor_reduce(v12[:, 1:2], c2,
                                    axis=mybir.AxisListType.X, op=ALU.add)

            # ---- partition-sum via ones-matmul, then DMA out ----
            pso = po.tile([1, 2], F32)
            nc.tensor.matmul(pso, ones, v12, start=True, stop=True)
            outsb = big.tile([1, 2], F32)
            nc.vector.tensor_copy(outsb, pso)
            nc.sync.dma_start(out=out_d, in_=outsb)

    nc.compile()
    return nc


_NC_CACHE = None


def _get_nc():
    global _NC_CACHE
    if _NC_CACHE is None:
        _NC_CACHE = build_program()
    return _NC_CACHE


def make_in_maps(z_i, z_j, probs, target):
    p = np.ascontiguousarray(
        np.concatenate([z_i, z_j], axis=0), dtype=np.float32)
    t2 = np.concatenate([target, target]).astype(np.float32)
    probs = np.asarray(probs, dtype=np.float32)
    iotah = np.broadcast_to(np.arange(NCLS, dtype=np.float32),
                            (128, NCLS)).copy()
    in_maps = []
    for k in range(NCORES):
        lo = k * R
        plo = (lo + N) % M
        # tgtrep[p, n, c] = t2[lo + n*128 + p] for all c
        tgt_k = t2[lo:lo + R].reshape(NS, 128).T          # [128, NS]
        tgtrep = np.ascontiguousarray(
            np.repeat(tgt_k[:, :, None], NCLS, axis=2), dtype=np.float32)
        in_maps.append({
            "p": p,
            "psq": np.ascontiguousarray(
                np.concatenate([p[lo:lo + R], p[plo:plo + R]], axis=0)),
            "probs": np.ascontiguousarray(probs[lo:lo + R]),
            "iotah": iotah,
            "tgtrep": tgtrep,
        })
    return in_maps


def kernel(z_i, z_j, probs, target, neg_idx):
    # neg_idx is the fixed structured NT-Xent mask (all columns except self and
    # positive); its effect is computed analytically, so it is never read.
    del neg_idx
    nc = _get_nc()
    in_maps = make_in_maps(np.asarray(z_i), np.asarray(z_j),
                           np.asarray(probs), np.asarray(target))
    res = run_bass_kernel_spmd(nc, in_maps, list(range(NCORES)))
    parts = np.stack([res.results[k]["out"].reshape(2) for k in range(NCORES)])
    total = parts.sum(axis=0) / np.float32(M)
    l1 = np.float32(total[0])
    l2 = np.float32(total[1])
    return (np.asarray(l1), np.asarray(l2))



# revision 2
# speedup vs baseline: 2.1172x; 2.1172x over previous
"""NT-Xent loss kernel for Trainium2, 8-core SPMD with on-device AllGather.

Math: with p = cat(z_i, z_j) [8192, 64], pn = p / max(||p||, 1e-8),
sim = 2 * pn @ pn.T (TEMP=0.5), the reference's gather-based losses reduce to
  loss1 = mean_r( log(sum_c exp(sim[r,c]) - exp(sim[r,r])) - pos_r )
  loss2 = mean_r( log(exp(pos_r) + sum_{c != t_r} exp(probs[r,c])) - pos_r )
where pos_r = sim[r, partner(r)].  sim entries lie in [-2, 2] so no max-shift
is needed.  The huge neg_idx input is a fixed structured mask and is never
read; the probs negative selection (drop own class) is 8192x10 index prep
done on host.

Both losses are sums of per-row terms, and each row term depends only on the
full column set -- so any symmetric permutation of the row order leaves them
unchanged.  We permute rows so that each core's 1024 rows are
[z_i[k*512:(k+1)*512]; z_j[k*512:(k+1)*512]]: every row's positive partner
lives on the same core at a fixed local offset (+-512 rows = chunk n <-> n+4),
which keeps the SPMD program branch-free.

Each core receives ONLY its own 1024 rows packed with its probs negatives in
one bf16 blob [1024, 73] (the wall-clock cost of a call is dominated by
per-array host->device transfer latency over the axon tunnel, so everything
rides in a single small array).  On device: normalize locally, transpose,
AllGather the transposed normalized rows (bf16, 128KB/core) across the 8
cores over the on-chip links, then compute the [1024 x 8192] slab of
exp(sim) and both loss tails.  Host sums the 8 partial pairs.

Dispatch: bass_utils.run_bass_kernel_spmd rebuilds its jax.jit on every call
(full retrace + relower + NEFF rewrap + executable reload over the tunnel,
~200ms) and ships per-core arrays individually (~70ms each).  The first
kernel() call runs through run_bass_kernel_spmd; it also builds a
semantically identical runner around the same _bass_exec_p primitive with
the jitted executable cached, verifies it against the stock result, and
subsequent calls dispatch through that.
"""

import numpy as np
import ml_dtypes

import concourse.bass as bass
import concourse.bacc as bacc
import concourse.tile as tile
from concourse import mybir
from concourse.masks import make_identity
from concourse.bass_utils import run_bass_kernel_spmd

N = 4096
D = 64
M = 2 * N            # 8192 rows of sim
NCORES = 8
R = M // NCORES      # 1024 rows per core
NS = R // 128        # 8 row-chunks of 128 per core
JJ = 8               # column groups of 1024 in the main loop
NCLS = 10
NNEG = NCLS - 1      # 9 probs negatives per row
PC = D + NNEG        # 73 blob columns
INV_TEMP = 2.0       # 1 / 0.5
F32 = mybir.dt.float32
BF16 = mybir.dt.bfloat16
NEWTON_ITERS = 2
N_ACC = 28           # of the 64 row-sum reductions, how many use ACT accum_out

AF = mybir.ActivationFunctionType
ALU = mybir.AluOpType


def _emit_rsqrt(nc, pool, n2, nchunk):
    """inv = 1/max(sqrt(n2), 1e-8) on DVE: quake-style magic-constant seed +
    Newton steps (keeps ACT exclusively on Exp/Ln so its table never
    thrashes)."""
    eng = nc.vector
    I32 = mybir.dt.int32
    inv = pool.tile([128, nchunk], F32, tag="rs_inv")
    eng.tensor_scalar(inv.bitcast(I32), n2.bitcast(I32), 1, None,
                      ALU.arith_shift_right)
    eng.tensor_scalar(inv.bitcast(I32), inv.bitcast(I32), -1, 0x5F3759DF,
                      ALU.mult, ALU.add)
    t2 = pool.tile([128, nchunk], F32, tag="rs_t2")
    for _ in range(NEWTON_ITERS):
        eng.tensor_mul(t2, inv, inv)
        eng.tensor_mul(t2, t2, n2)
        eng.tensor_scalar(t2, t2, -0.5, 1.5, ALU.mult, ALU.add)
        eng.tensor_mul(inv, inv, t2)
    eng.tensor_scalar_min(inv, inv, 1e8)
    return inv


def build_program():
    nc = bacc.Bacc("TRN2", target_bir_lowering=False, debug=False,
                   num_devices=NCORES)

    blob_d = nc.dram_tensor("blob", [R, PC], BF16, kind="ExternalInput").ap()
    out_d = nc.dram_tensor("out", [1, 2], F32, kind="ExternalOutput").ap()

    with tile.TileContext(nc) as tc:
        with tc.tile_pool(name="consts", bufs=1) as consts, \
             tc.tile_pool(name="big", bufs=1) as big, \
             tc.tile_pool(name="work", bufs=2) as work, \
             tc.tile_pool(name="tp", bufs=2, space="PSUM") as tp, \
             tc.tile_pool(name="mm", bufs=2, space="PSUM") as mm, \
             tc.tile_pool(name="po", bufs=1, space="PSUM") as po, \
             tc.tile_pool(name="esc", bufs=6) as esc, \
             tc.tile_pool(name="dram", bufs=1, space="DRAM") as dram:

            identity = consts.tile([128, 128], BF16)
            make_identity(nc, identity)
            ones = consts.tile([128, 1], F32)
            nc.vector.memset(ones, 1.0)

            # ---- load the packed blob ----
            braw = big.tile([128, NS, PC], BF16)
            nc.sync.dma_start(
                out=braw, in_=blob_d.rearrange("(n p) c -> p n c", p=128))
            praw = braw[:, :, 0:D]          # [128, 8, 64] bf16 (strided)
            pneg = braw[:, :, D:PC]         # [128, 8, 9]  bf16 (strided)

            # ---- normalize local rows ----
            sq = big.tile([128, NS, D], F32)
            nc.vector.tensor_mul(sq, praw, praw)
            n2 = big.tile([128, NS], F32)
            nc.vector.tensor_reduce(n2, sq, axis=mybir.AxisListType.X,
                                    op=ALU.add)
            inv = _emit_rsqrt(nc, big, n2, NS)
            pnb = big.tile([128, NS, D], BF16)
            for n in range(NS):
                nc.vector.tensor_scalar_mul(pnb[:, n, :], praw[:, n, :],
                                            inv[:, n:n + 1])

            # ---- transpose local pn -> psT [64, 1024], kick off AllGather ----
            psT = big.tile([64, R], BF16)
            for q4 in range(2):
                tpp = tp.tile([64, 512], BF16, tag="tp")
                for q in range(4):
                    nn = 4 * q4 + q
                    nc.tensor.transpose(tpp[:, q * 128:(q + 1) * 128],
                                        pnb[:, nn, :], identity)
                nc.vector.tensor_copy(psT[:, q4 * 512:(q4 + 1) * 512], tpp)

            inb = dram.tile([64, R], BF16)
            outb = dram.tile([NCORES, 64, R], BF16)
            nc.gpsimd.dma_start(inb[:], psT[:])
            nc.gpsimd.collective_compute(
                "AllGather", ALU.bypass,
                replica_groups=[list(range(NCORES))],
                ins=[inb.opt()], outs=[outb.opt()],
            )
            pnT = big.tile([64, NCORES, R], BF16)
            nc.sync.dma_start(out=pnT, in_=outb.rearrange("k d c -> d k c"))
            pnTf = pnT.rearrange("d k c -> d (k c)")      # [64, 8192]

            # ---- probs negatives + pos/diag (overlaps the AllGather) ----
            pnegc = big.tile([128, NS, NNEG], F32)
            nc.vector.tensor_copy(pnegc, pneg)
            eprobs = big.tile([128, NS, NNEG], F32)
            nc.scalar.activation(eprobs.rearrange("p n c -> p (n c)"),
                                 pnegc.rearrange("p n c -> p (n c)"), AF.Exp)
            ps2 = big.tile([128, NS], F32)
            nc.vector.tensor_reduce(ps2, eprobs, axis=mybir.AxisListType.X,
                                    op=ALU.add)

            dq = work.tile([128, NS, D], F32, tag="rowdot")
            nc.vector.tensor_mul(dq, pnb, pnb)
            diag_raw = big.tile([128, NS], F32)
            nc.vector.tensor_reduce(diag_raw, dq, axis=mybir.AxisListType.X,
                                    op=ALU.add)
            # positive partner of chunk n is chunk (n+4)%8, same partition
            ph = work.tile([128, 4, D], F32, tag="rowdot")
            nc.vector.tensor_mul(ph, pnb[:, 0:4, :], pnb[:, 4:8, :])
            pos_raw = big.tile([128, NS], F32)
            nc.vector.tensor_reduce(pos_raw[:, 0:4], ph,
                                    axis=mybir.AxisListType.X, op=ALU.add)
            nc.vector.tensor_copy(pos_raw[:, 4:8], pos_raw[:, 0:4])

            ediag = big.tile([128, NS], F32)
            nc.scalar.activation(ediag, diag_raw, AF.Exp, scale=INV_TEMP)
            epos = big.tile([128, NS], F32)
            nc.scalar.activation(epos, pos_raw, AF.Exp, scale=INV_TEMP)
            pos2 = big.tile([128, NS], F32)
            nc.vector.tensor_scalar_mul(pos2, pos_raw, INV_TEMP)

            # ---- main loop: exp(sim slab) row sums ----
            scols = big.tile([128, NS * JJ], F32)
            for jj in range(JJ):
                c0 = jj * 1024
                for n in range(NS):
                    idx = n * JJ + jj
                    pst = mm.tile([128, 1024], F32, tag="mm")
                    lhsT = psT[:, n * 128:(n + 1) * 128]
                    nc.tensor.matmul(pst[:, 0:512], lhsT,
                                     pnTf[:, c0:c0 + 512],
                                     start=True, stop=True)
                    nc.tensor.matmul(pst[:, 512:1024], lhsT,
                                     pnTf[:, c0 + 512:c0 + 1024],
                                     start=True, stop=True)
                    et = esc.tile([128, 1024], F32, tag="esc")
                    if (idx * N_ACC) % (NS * JJ) < N_ACC:
                        nc.scalar.activation(
                            et, pst, AF.Exp, scale=INV_TEMP,
                            accum_out=scols[:, idx:idx + 1])
                    else:
                        nc.scalar.activation(et, pst, AF.Exp,
                                             scale=INV_TEMP)
                        nc.vector.tensor_reduce(
                            scols[:, idx:idx + 1], et,
                            axis=mybir.AxisListType.X, op=ALU.add)

            # ---- loss tails ----
            stot = big.tile([128, NS], F32)
            nc.vector.tensor_reduce(
                stot, scols.rearrange("p (n j) -> p n j", j=JJ),
                axis=mybir.AxisListType.X, op=ALU.add)
            s1 = big.tile([128, NS], F32)
            nc.vector.tensor_sub(s1, stot, ediag)
            lse1 = big.tile([128, NS], F32)
            nc.scalar.activation(lse1, s1, AF.Ln)
            c1 = big.tile([128, NS], F32)
            nc.vector.tensor_sub(c1, lse1, pos2)
            v12 = big.tile([128, 2], F32)
            nc.vector.tensor_reduce(v12[:, 0:1], c1,
                                    axis=mybir.AxisListType.X, op=ALU.add)

            s2 = big.tile([128, NS], F32)
            nc.vector.tensor_add(s2, ps2, epos)
            # false data-dep on stot so the scheduler cannot hoist the Ln
            # into the exp stream (each hoist costs 2 ACT table swaps)
            nc.vector.scalar_tensor_tensor(
                out=s2, in0=stot, scalar=0.0, in1=s2,
                op0=ALU.mult, op1=ALU.add)
            lse2 = big.tile([128, NS], F32)
            nc.scalar.activation(lse2, s2, AF.Ln)
            c2 = big.tile([128, NS], F32)
            nc.vector.tensor_sub(c2, lse2, pos2)
            nc.vector.tensor_reduce(v12[:, 1:2], c2,
                                    axis=mybir.AxisListType.X, op=ALU.add)

            # ---- partition-sum via ones-matmul, then DMA out ----
            pso = po.tile([1, 2], F32)
            nc.tensor.matmul(pso, ones, v12, start=True, stop=True)
            outsb = big.tile([1, 2], F32)
            nc.vector.tensor_copy(outsb, pso)
            nc.sync.dma_start(out=out_d, in_=outsb)

    nc.compile()
    return nc


_NC_CACHE = None


def _get_nc():
    global _NC_CACHE
    if _NC_CACHE is None:
        _NC_CACHE = build_program()
    return _NC_CACHE


def make_blob(z_i, z_j, probs, target):
    """[8, 1024, 73] bf16: per shard k rows [z_i[k*512:(k+1)*512];
    z_j[...]], cols 0:64 = p row, 64:73 = probs with own class dropped."""
    z_i = np.asarray(z_i, np.float32)
    z_j = np.asarray(z_j, np.float32)
    probs = np.asarray(probs, np.float32)
    t2 = np.concatenate([np.asarray(target), np.asarray(target)])
    keep = np.arange(NCLS)[None, :] != t2[:, None]
    pn9 = probs[keep].reshape(M, NNEG)
    blob = np.empty((NCORES, R, PC), ml_dtypes.bfloat16)
    half = R // 2
    blob[:, :half, :D] = z_i.reshape(NCORES, half, D)
    blob[:, half:, :D] = z_j.reshape(NCORES, half, D)
    blob[:, :half, D:] = pn9[:N].reshape(NCORES, half, NNEG)
    blob[:, half:, D:] = pn9[N:].reshape(NCORES, half, NNEG)
    return blob


def make_in_maps(z_i, z_j, probs, target):
    blob = make_blob(z_i, z_j, probs, target)
    return [{"blob": blob[k]} for k in range(NCORES)]


def _assemble(results):
    parts = np.stack([results[k]["out"].reshape(2) for k in range(NCORES)])
    total = parts.sum(axis=0) / np.float32(M)
    return (np.asarray(np.float32(total[0])), np.asarray(np.float32(total[1])))


class _CachedRunner:
    """run_bass_via_pjrt with the jitted executable built once and the
    donated output zeros created on device (each host->device array costs
    ~70ms of tunnel latency, so per-call traffic is 1 input array)."""

    def __init__(self, nc, n_cores):
        import jax
        import jax.numpy as jnp
        from jax.sharding import Mesh, PartitionSpec, NamedSharding
        try:
            from jax import shard_map
        except ImportError:
            from jax.experimental.shard_map import shard_map
        from concourse import bass2jax

        bass2jax.install_neuronx_cc_hook()
        self._jax = jax
        self._np = np
        partition_name = (nc.partition_id_tensor.name
                          if nc.partition_id_tensor else None)

        in_names, out_names, out_avals, zero_shapes = [], [], [], []
        for alloc in nc.m.functions[0].allocations:
            if not isinstance(alloc, mybir.MemoryLocationSet):
                continue
            name = alloc.memorylocations[0].name
            if alloc.kind == "ExternalInput":
                if name != partition_name:
                    in_names.append(name)
            elif alloc.kind == "ExternalOutput":
                out_names.append(name)
                shape = tuple(alloc.tensor_shape)
                dtype = mybir.dt.np(alloc.dtype)
                out_avals.append(jax.core.ShapedArray(shape, dtype))
                zero_shapes.append((shape, dtype))
        n_params = len(in_names)
        n_outs = len(out_avals)
        all_in_names = list(in_names) + list(out_names)
        if partition_name is not None:
            all_in_names.append(partition_name)
        donate = tuple(range(n_params, n_params + n_outs))
        self._in_names = in_names
        self._out_names = out_names
        self._out_avals = out_avals
        self._n_cores = n_cores

        def _body(*args):
            operands = list(args)
            if partition_name is not None:
                operands.append(bass2jax.partition_id_tensor())
            outs = bass2jax._bass_exec_p.bind(
                *operands,
                out_avals=tuple(out_avals),
                in_names=tuple(all_in_names),
                out_names=tuple(out_names),
                lowering_input_output_aliases=(),
                sim_require_finite=True,
                sim_require_nnan=True,
                nc=nc,
            )
            return tuple(outs)

        devices = jax.devices()[:n_cores]
        mesh = Mesh(np.asarray(devices), ("core",))
        in_specs = (PartitionSpec("core"),) * (n_params + n_outs)
        out_specs = (PartitionSpec("core"),) * len(out_names)
        self._sharded = jax.jit(
            shard_map(_body, mesh=mesh, in_specs=in_specs,
                      out_specs=out_specs, check_rep=False),
            donate_argnums=donate, keep_unused=True,
        )
        csh = NamedSharding(mesh, PartitionSpec("core"))

        def _zeros():
            return tuple(
                jnp.zeros((n_cores * s[0], *s[1:]), d)
                for s, d in zero_shapes)

        self._zf = jax.jit(_zeros, out_shardings=(csh,) * n_outs)

    def run(self, in_maps):
        np_ = self._np
        per_core = [[np_.asarray(m[name]) for name in self._in_names]
                    for m in in_maps]
        concat_in = [
            np_.concatenate([per_core[c][i] for c in range(self._n_cores)],
                            axis=0)
            for i in range(len(self._in_names))
        ]
        zeros = self._zf()
        out_arrs = self._sharded(*concat_in, *zeros)
        return [
            {
                name: np_.asarray(out_arrs[i]).reshape(
                    self._n_cores, *self._out_avals[i].shape)[c]
                for i, name in enumerate(self._out_names)
            }
            for c in range(self._n_cores)
        ]


_RUNNER = None
_RUNNER_FAILED = False


def kernel(z_i, z_j, probs, target, neg_idx):
    # neg_idx is the fixed structured NT-Xent mask (all columns except self
    # and positive); its effect is computed analytically, so it's never read.
    del neg_idx
    global _RUNNER, _RUNNER_FAILED
    nc = _get_nc()
    in_maps = make_in_maps(z_i, z_j, probs, target)

    if _RUNNER is not None:
        return _assemble(_RUNNER.run(in_maps))

    res = run_bass_kernel_spmd(nc, in_maps, list(range(NCORES)))
    out = _assemble(res.results)

    if not _RUNNER_FAILED:
        try:
            runner = _CachedRunner(nc, NCORES)
            chk = _assemble(runner.run(in_maps))
            if (abs(float(chk[0]) - float(out[0])) <= 1e-4 * abs(float(out[0]))
                    and abs(float(chk[1]) - float(out[1]))
                    <= 1e-4 * abs(float(out[1]))):
                _RUNNER = runner
            else:
                _RUNNER_FAILED = True
        except Exception:
            _RUNNER_FAILED = True
    return out


# revision 3
# speedup vs baseline: 8.6353x; 4.0787x over previous
"""NT-Xent loss kernel for Trainium2, 8-core SPMD with on-device AllGather.

Math: with p = cat(z_i, z_j) [8192, 64], pn = p / max(||p||, 1e-8),
sim = 2 * pn @ pn.T (TEMP=0.5), the reference's gather-based losses reduce to
  loss1 = mean_r( log(sum_c exp(sim[r,c]) - exp(sim[r,r])) - pos_r )
  loss2 = mean_r( log(exp(pos_r) + sum_{c != t_r} exp(probs[r,c])) - pos_r )
where pos_r = sim[r, partner(r)].  sim entries lie in [-2, 2] so no max-shift
is needed.  The huge neg_idx input is a fixed structured mask and is never
read; the probs negative selection (drop own class) is 8192x10 index prep
done on host.

Both losses are sums of per-row terms, and each row term depends only on the
full column set -- so any symmetric permutation of the row order leaves them
unchanged.  We permute rows so that each core's 1024 rows are
[z_i[k*512:(k+1)*512]; z_j[k*512:(k+1)*512]]: every row's positive partner
lives on the same core at a fixed local offset (+-512 rows = chunk n <-> n+4),
which keeps the SPMD program branch-free.

Each core receives ONLY its own 1024 rows packed with its probs negatives in
one bf16 blob [1024, 73] (the wall-clock cost of a call is dominated by
per-array host->device transfer latency over the axon tunnel, so everything
rides in a single small array).  On device: normalize locally, transpose,
AllGather the transposed normalized rows (bf16, 128KB/core) across the 8
cores over the on-chip links, then compute the [1024 x 8192] slab of
exp(sim) and both loss tails.  Host sums the 8 partial pairs.

Dispatch: bass_utils.run_bass_kernel_spmd rebuilds its jax.jit on every call
(full retrace + relower + NEFF rewrap + executable reload over the tunnel,
~200ms) and ships per-core arrays individually (~70ms each).  The first
kernel() call runs through run_bass_kernel_spmd; it also builds a
semantically identical runner around the same _bass_exec_p primitive with
the jitted executable cached, verifies it against the stock result, and
subsequent calls dispatch through that.
"""

import numpy as np
import ml_dtypes

import concourse.bass as bass
import concourse.bacc as bacc
import concourse.tile as tile
from concourse import mybir
from concourse.masks import make_identity
from concourse.bass_utils import run_bass_kernel_spmd

N = 4096
D = 64
M = 2 * N            # 8192 rows of sim
NCORES = 8
R = M // NCORES      # 1024 rows per core
NS = R // 128        # 8 row-chunks of 128 per core
JJ = 8               # column groups of 1024 in the main loop
NCLS = 10
NNEG = NCLS - 1      # 9 probs negatives per row
PC = D + NNEG        # 73 blob columns
INV_TEMP = 2.0       # 1 / 0.5
F32 = mybir.dt.float32
BF16 = mybir.dt.bfloat16
NEWTON_ITERS = 2
N_ACC = 28           # of the 64 row-sum reductions, how many use ACT accum_out

AF = mybir.ActivationFunctionType
ALU = mybir.AluOpType


def _emit_rsqrt(nc, pool, n2, nchunk):
    """inv = 1/max(sqrt(n2), 1e-8) on DVE: quake-style magic-constant seed +
    Newton steps (keeps ACT exclusively on Exp/Ln so its table never
    thrashes)."""
    eng = nc.vector
    I32 = mybir.dt.int32
    inv = pool.tile([128, nchunk], F32, tag="rs_inv")
    eng.tensor_scalar(inv.bitcast(I32), n2.bitcast(I32), 1, None,
                      ALU.arith_shift_right)
    eng.tensor_scalar(inv.bitcast(I32), inv.bitcast(I32), -1, 0x5F3759DF,
                      ALU.mult, ALU.add)
    t2 = pool.tile([128, nchunk], F32, tag="rs_t2")
    for _ in range(NEWTON_ITERS):
        eng.tensor_mul(t2, inv, inv)
        eng.tensor_mul(t2, t2, n2)
        eng.tensor_scalar(t2, t2, -0.5, 1.5, ALU.mult, ALU.add)
        eng.tensor_mul(inv, inv, t2)
    eng.tensor_scalar_min(inv, inv, 1e8)
    return inv


def build_program():
    nc = bacc.Bacc("TRN2", target_bir_lowering=False, debug=False,
                   num_devices=NCORES)

    blob_d = nc.dram_tensor("blob", [R, PC], BF16, kind="ExternalInput").ap()
    out_d = nc.dram_tensor("out", [1, 2], F32, kind="ExternalOutput").ap()

    with tile.TileContext(nc) as tc:
        with tc.tile_pool(name="consts", bufs=1) as consts, \
             tc.tile_pool(name="big", bufs=1) as big, \
             tc.tile_pool(name="work", bufs=2) as work, \
             tc.tile_pool(name="tp", bufs=2, space="PSUM") as tp, \
             tc.tile_pool(name="mm", bufs=2, space="PSUM") as mm, \
             tc.tile_pool(name="po", bufs=1, space="PSUM") as po, \
             tc.tile_pool(name="esc", bufs=6) as esc, \
             tc.tile_pool(name="dram", bufs=1, space="DRAM") as dram:

            identity = consts.tile([128, 128], BF16)
            make_identity(nc, identity)
            ones = consts.tile([128, 1], F32)
            nc.vector.memset(ones, 1.0)

            # ---- load the packed blob ----
            braw = big.tile([128, NS, PC], BF16)
            nc.sync.dma_start(
                out=braw, in_=blob_d.rearrange("(n p) c -> p n c", p=128))
            praw = braw[:, :, 0:D]          # [128, 8, 64] bf16 (strided)
            pneg = braw[:, :, D:PC]         # [128, 8, 9]  bf16 (strided)

            # ---- normalize local rows ----
            sq = big.tile([128, NS, D], F32)
            nc.vector.tensor_mul(sq, praw, praw)
            n2 = big.tile([128, NS], F32)
            nc.vector.tensor_reduce(n2, sq, axis=mybir.AxisListType.X,
                                    op=ALU.add)
            inv = _emit_rsqrt(nc, big, n2, NS)
            pnb = big.tile([128, NS, D], BF16)
            for n in range(NS):
                nc.vector.tensor_scalar_mul(pnb[:, n, :], praw[:, n, :],
                                            inv[:, n:n + 1])

            # ---- transpose local pn -> psT [64, 1024], kick off AllGather ----
            psT = big.tile([64, R], BF16)
            for q4 in range(2):
                tpp = tp.tile([64, 512], BF16, tag="tp")
                for q in range(4):
                    nn = 4 * q4 + q
                    nc.tensor.transpose(tpp[:, q * 128:(q + 1) * 128],
                                        pnb[:, nn, :], identity)
                nc.vector.tensor_copy(psT[:, q4 * 512:(q4 + 1) * 512], tpp)

            inb = dram.tile([64, R], BF16)
            outb = dram.tile([NCORES, 64, R], BF16)
            nc.gpsimd.dma_start(inb[:], psT[:])
            nc.gpsimd.collective_compute(
                "AllGather", ALU.bypass,
                replica_groups=[list(range(NCORES))],
                ins=[inb.opt()], outs=[outb.opt()],
            )
            pnT = big.tile([64, NCORES, R], BF16)
            nc.sync.dma_start(out=pnT, in_=outb.rearrange("k d c -> d k c"))
            pnTf = pnT.rearrange("d k c -> d (k c)")      # [64, 8192]

            # ---- probs negatives + pos/diag (overlaps the AllGather) ----
            pnegc = big.tile([128, NS, NNEG], F32)
            nc.vector.tensor_copy(pnegc, pneg)
            eprobs = big.tile([128, NS, NNEG], F32)
            nc.scalar.activation(eprobs.rearrange("p n c -> p (n c)"),
                                 pnegc.rearrange("p n c -> p (n c)"), AF.Exp)
            ps2 = big.tile([128, NS], F32)
            nc.vector.tensor_reduce(ps2, eprobs, axis=mybir.AxisListType.X,
                                    op=ALU.add)

            dq = work.tile([128, NS, D], F32, tag="rowdot")
            nc.vector.tensor_mul(dq, pnb, pnb)
            diag_raw = big.tile([128, NS], F32)
            nc.vector.tensor_reduce(diag_raw, dq, axis=mybir.AxisListType.X,
                                    op=ALU.add)
            # positive partner of chunk n is chunk (n+4)%8, same partition
            ph = work.tile([128, 4, D], F32, tag="rowdot")
            nc.vector.tensor_mul(ph, pnb[:, 0:4, :], pnb[:, 4:8, :])
            pos_raw = big.tile([128, NS], F32)
            nc.vector.tensor_reduce(pos_raw[:, 0:4], ph,
                                    axis=mybir.AxisListType.X, op=ALU.add)
            nc.vector.tensor_copy(pos_raw[:, 4:8], pos_raw[:, 0:4])

            ediag = big.tile([128, NS], F32)
            nc.scalar.activation(ediag, diag_raw, AF.Exp, scale=INV_TEMP)
            epos = big.tile([128, NS], F32)
            nc.scalar.activation(epos, pos_raw, AF.Exp, scale=INV_TEMP)
            pos2 = big.tile([128, NS], F32)
            nc.vector.tensor_scalar_mul(pos2, pos_raw, INV_TEMP)

            # ---- main loop: exp(sim slab) row sums ----
            scols = big.tile([128, NS * JJ], F32)
            for jj in range(JJ):
                c0 = jj * 1024
                for n in range(NS):
                    idx = n * JJ + jj
                    pst = mm.tile([128, 1024], F32, tag="mm")
                    lhsT = psT[:, n * 128:(n + 1) * 128]
                    nc.tensor.matmul(pst[:, 0:512], lhsT,
                                     pnTf[:, c0:c0 + 512],
                                     start=True, stop=True)
                    nc.tensor.matmul(pst[:, 512:1024], lhsT,
                                     pnTf[:, c0 + 512:c0 + 1024],
                                     start=True, stop=True)
                    et = esc.tile([128, 1024], F32, tag="esc")
                    if (idx * N_ACC) % (NS * JJ) < N_ACC:
                        nc.scalar.activation(
                            et, pst, AF.Exp, scale=INV_TEMP,
                            accum_out=scols[:, idx:idx + 1])
                    else:
                        nc.scalar.activation(et, pst, AF.Exp,
                                             scale=INV_TEMP)
                        nc.vector.tensor_reduce(
                            scols[:, idx:idx + 1], et,
                            axis=mybir.AxisListType.X, op=ALU.add)

            # ---- loss tails ----
            stot = big.tile([128, NS], F32)
            nc.vector.tensor_reduce(
                stot, scols.rearrange("p (n j) -> p n j", j=JJ),
                axis=mybir.AxisListType.X, op=ALU.add)
            s1 = big.tile([128, NS], F32)
            nc.vector.tensor_sub(s1, stot, ediag)
            lse1 = big.tile([128, NS], F32)
            nc.scalar.activation(lse1, s1, AF.Ln)
            c1 = big.tile([128, NS], F32)
            nc.vector.tensor_sub(c1, lse1, pos2)
            v12 = big.tile([128, 2], F32)
            nc.vector.tensor_reduce(v12[:, 0:1], c1,
                                    axis=mybir.AxisListType.X, op=ALU.add)

            s2 = big.tile([128, NS], F32)
            nc.vector.tensor_add(s2, ps2, epos)
            # false data-dep on stot so the scheduler cannot hoist the Ln
            # into the exp stream (each hoist costs 2 ACT table swaps)
            nc.vector.scalar_tensor_tensor(
                out=s2, in0=stot, scalar=0.0, in1=s2,
                op0=ALU.mult, op1=ALU.add)
            lse2 = big.tile([128, NS], F32)
            nc.scalar.activation(lse2, s2, AF.Ln)
            c2 = big.tile([128, NS], F32)
            nc.vector.tensor_sub(c2, lse2, pos2)
            nc.vector.tensor_reduce(v12[:, 1:2], c2,
                                    axis=mybir.AxisListType.X, op=ALU.add)

            # ---- partition-sum via ones-matmul, then DMA out ----
            pso = po.tile([1, 2], F32)
            nc.tensor.matmul(pso, ones, v12, start=True, stop=True)
            outsb = big.tile([1, 2], F32)
            nc.vector.tensor_copy(outsb, pso)
            nc.sync.dma_start(out=out_d, in_=outsb)

    nc.compile()
    return nc


_NC_CACHE = None


def _get_nc():
    global _NC_CACHE
    if _NC_CACHE is None:
        _NC_CACHE = build_program()
    return _NC_CACHE


def make_blob(z_i, z_j, probs, target):
    """[8, 1024, 73] bf16: per shard k rows [z_i[k*512:(k+1)*512];
    z_j[...]], cols 0:64 = p row, 64:73 = probs with own class dropped."""
    z_i = np.asarray(z_i, np.float32)
    z_j = np.asarray(z_j, np.float32)
    probs = np.asarray(probs, np.float32)
    t2 = np.concatenate([np.asarray(target), np.asarray(target)])
    keep = np.arange(NCLS)[None, :] != t2[:, None]
    pn9 = probs[keep].reshape(M, NNEG)
    blob = np.empty((NCORES, R, PC), ml_dtypes.bfloat16)
    half = R // 2
    blob[:, :half, :D] = z_i.reshape(NCORES, half, D)
    blob[:, half:, :D] = z_j.reshape(NCORES, half, D)
    blob[:, :half, D:] = pn9[:N].reshape(NCORES, half, NNEG)
    blob[:, half:, D:] = pn9[N:].reshape(NCORES, half, NNEG)
    return blob


def make_in_maps(z_i, z_j, probs, target):
    blob = make_blob(z_i, z_j, probs, target)
    return [{"blob": blob[k]} for k in range(NCORES)]


def _assemble(results):
    parts = np.stack([results[k]["out"].reshape(2) for k in range(NCORES)])
    total = parts.sum(axis=0) / np.float32(M)
    return (np.asarray(np.float32(total[0])), np.asarray(np.float32(total[1])))


class _CachedRunner:
    """run_bass_via_pjrt with the jitted executable built once and the
    donated output zeros created on device (each host->device array costs
    ~70ms of tunnel latency, so per-call traffic is 1 input array)."""

    def __init__(self, nc, n_cores):
        import jax
        import jax.numpy as jnp
        from jax.sharding import Mesh, PartitionSpec, NamedSharding
        import warnings
        with warnings.catch_warnings():
            warnings.simplefilter("ignore")
            from jax.experimental.shard_map import shard_map
        from concourse import bass2jax

        bass2jax.install_neuronx_cc_hook()
        self._jax = jax
        self._np = np
        partition_name = (nc.partition_id_tensor.name
                          if nc.partition_id_tensor else None)

        in_names, out_names, out_avals, zero_shapes = [], [], [], []
        for alloc in nc.m.functions[0].allocations:
            if not isinstance(alloc, mybir.MemoryLocationSet):
                continue
            name = alloc.memorylocations[0].name
            if alloc.kind == "ExternalInput":
                if name != partition_name:
                    in_names.append(name)
            elif alloc.kind == "ExternalOutput":
                out_names.append(name)
                shape = tuple(alloc.tensor_shape)
                dtype = mybir.dt.np(alloc.dtype)
                out_avals.append(jax.core.ShapedArray(shape, dtype))
                zero_shapes.append((shape, dtype))
        n_params = len(in_names)
        n_outs = len(out_avals)
        all_in_names = list(in_names) + list(out_names)
        if partition_name is not None:
            all_in_names.append(partition_name)
        donate = tuple(range(n_params, n_params + n_outs))
        self._in_names = in_names
        self._out_names = out_names
        self._out_avals = out_avals
        self._n_cores = n_cores

        def _body(*args):
            operands = list(args)
            if partition_name is not None:
                operands.append(bass2jax.partition_id_tensor())
            outs = bass2jax._bass_exec_p.bind(
                *operands,
                out_avals=tuple(out_avals),
                in_names=tuple(all_in_names),
                out_names=tuple(out_names),
                lowering_input_output_aliases=(),
                sim_require_finite=True,
                sim_require_nnan=True,
                nc=nc,
            )
            return tuple(outs)

        devices = jax.devices()[:n_cores]
        mesh = Mesh(np.asarray(devices), ("core",))
        in_specs = (PartitionSpec("core"),) * (n_params + n_outs)
        out_specs = (PartitionSpec("core"),) * len(out_names)
        self._sharded = jax.jit(
            shard_map(_body, mesh=mesh, in_specs=in_specs,
                      out_specs=out_specs, check_rep=False),
            donate_argnums=donate, keep_unused=True,
        )
        csh = NamedSharding(mesh, PartitionSpec("core"))

        def _zeros():
            return tuple(
                jnp.zeros((n_cores * s[0], *s[1:]), d)
                for s, d in zero_shapes)

        self._zf = jax.jit(_zeros, out_shardings=(csh,) * n_outs)

    def run(self, in_maps):
        np_ = self._np
        per_core = [[np_.asarray(m[name]) for name in self._in_names]
                    for m in in_maps]
        concat_in = [
            np_.concatenate([per_core[c][i] for c in range(self._n_cores)],
                            axis=0)
            for i in range(len(self._in_names))
        ]
        zeros = self._zf()
        out_arrs = self._sharded(*concat_in, *zeros)
        return [
            {
                name: np_.asarray(out_arrs[i]).reshape(
                    self._n_cores, *self._out_avals[i].shape)[c]
                for i, name in enumerate(self._out_names)
            }
            for c in range(self._n_cores)
        ]


_RUNNER = None
_RUNNER_FAILED = False


def kernel(z_i, z_j, probs, target, neg_idx):
    # neg_idx is the fixed structured NT-Xent mask (all columns except self
    # and positive); its effect is computed analytically, so it's never read.
    del neg_idx
    global _RUNNER, _RUNNER_FAILED
    nc = _get_nc()
    in_maps = make_in_maps(z_i, z_j, probs, target)

    if _RUNNER is not None:
        return _assemble(_RUNNER.run(in_maps))

    res = run_bass_kernel_spmd(nc, in_maps, list(range(NCORES)))
    out = _assemble(res.results)

    if not _RUNNER_FAILED:
        try:
            runner = _CachedRunner(nc, NCORES)
            chk = _assemble(runner.run(in_maps))
            if (abs(float(chk[0]) - float(out[0])) <= 1e-4 * abs(float(out[0]))
                    and abs(float(chk[1]) - float(out[1]))
                    <= 1e-4 * abs(float(out[1]))):
                _RUNNER = runner
            else:
                _RUNNER_FAILED = True
        except Exception:
            _RUNNER_FAILED = True
    return out


# revision 4
# speedup vs baseline: 9.2061x; 1.0661x over previous
"""NT-Xent loss kernel for Trainium2, 8-core SPMD with on-device AllGather.

Math: with p = cat(z_i, z_j) [8192, 64], pn = p / max(||p||, 1e-8),
sim = 2 * pn @ pn.T (TEMP=0.5), the reference's gather-based losses reduce to
  loss1 = mean_r( log(sum_c exp(sim[r,c]) - exp(sim[r,r])) - pos_r )
  loss2 = mean_r( log(exp(pos_r) + sum_{c != t_r} exp(probs[r,c])) - pos_r )
where pos_r = sim[r, partner(r)].  sim entries lie in [-2, 2] so no max-shift
is needed.  The huge neg_idx input is a fixed structured mask and is never
read; the probs negative selection (drop own class) is 8192x10 index prep
done on host.

Both losses are sums of per-row terms, and each row term depends only on the
full column set -- so any symmetric permutation of the row order leaves them
unchanged.  We permute rows so that each core's 1024 rows are
[z_i[k*512:(k+1)*512]; z_j[k*512:(k+1)*512]]: every row's positive partner
lives on the same core at a fixed local offset (+-512 rows = chunk n <-> n+4),
which keeps the SPMD program branch-free.

Each core receives ONLY its own 1024 rows packed with its probs negatives in
one bf16 blob [1024, 73] (the wall-clock cost of a call is dominated by
per-array host->device transfer latency over the axon tunnel, so everything
rides in a single small array).  On device: normalize locally, transpose,
AllGather the transposed normalized rows (bf16, 128KB/core) across the 8
cores over the on-chip links, then compute the [1024 x 8192] slab of
exp(sim) and both loss tails.  Host sums the 8 partial pairs.

Dispatch: bass_utils.run_bass_kernel_spmd rebuilds its jax.jit on every call
(full retrace + relower + NEFF rewrap + executable reload over the tunnel,
~200ms) and ships per-core arrays individually (~70ms each).  The first
kernel() call runs through run_bass_kernel_spmd; it also builds a
semantically identical runner around the same _bass_exec_p primitive with
the jitted executable cached, verifies it against the stock result, and
subsequent calls dispatch through that.
"""

import numpy as np
import ml_dtypes

import concourse.bass as bass
import concourse.bacc as bacc
import concourse.tile as tile
from concourse import mybir
from concourse.masks import make_identity
from concourse.bass_utils import run_bass_kernel_spmd

N = 4096
D = 64
M = 2 * N            # 8192 rows of sim
NCORES = 8
R = M // NCORES      # 1024 rows per core
NS = R // 128        # 8 row-chunks of 128 per core
JJ = 8               # column groups of 1024 in the main loop
NCLS = 10
NNEG = NCLS - 1      # 9 probs negatives per row
PC = D + NNEG        # 73 blob columns
INV_TEMP = 2.0       # 1 / 0.5
F32 = mybir.dt.float32
BF16 = mybir.dt.bfloat16
NEWTON_ITERS = 2
N_ACC = 28           # of the 64 row-sum reductions, how many use ACT accum_out

AF = mybir.ActivationFunctionType
ALU = mybir.AluOpType


def _emit_rsqrt(nc, pool, n2, nchunk):
    """inv = 1/max(sqrt(n2), 1e-8) on DVE: quake-style magic-constant seed +
    Newton steps (keeps ACT exclusively on Exp/Ln so its table never
    thrashes)."""
    eng = nc.vector
    I32 = mybir.dt.int32
    inv = pool.tile([128, nchunk], F32, tag="rs_inv")
    eng.tensor_scalar(inv.bitcast(I32), n2.bitcast(I32), 1, None,
                      ALU.arith_shift_right)
    eng.tensor_scalar(inv.bitcast(I32), inv.bitcast(I32), -1, 0x5F3759DF,
                      ALU.mult, ALU.add)
    t2 = pool.tile([128, nchunk], F32, tag="rs_t2")
    for _ in range(NEWTON_ITERS):
        eng.tensor_mul(t2, inv, inv)
        eng.tensor_mul(t2, t2, n2)
        eng.tensor_scalar(t2, t2, -0.5, 1.5, ALU.mult, ALU.add)
        eng.tensor_mul(inv, inv, t2)
    eng.tensor_scalar_min(inv, inv, 1e8)
    return inv


def build_program():
    nc = bacc.Bacc("TRN2", target_bir_lowering=False, debug=False,
                   num_devices=NCORES)

    blob_d = nc.dram_tensor("blob", [R, PC], BF16, kind="ExternalInput").ap()
    out_d = nc.dram_tensor("out", [1, 2], F32, kind="ExternalOutput").ap()

    with tile.TileContext(nc) as tc:
        with tc.tile_pool(name="consts", bufs=1) as consts, \
             tc.tile_pool(name="big", bufs=1) as big, \
             tc.tile_pool(name="work", bufs=2) as work, \
             tc.tile_pool(name="tp", bufs=2, space="PSUM") as tp, \
             tc.tile_pool(name="mm", bufs=2, space="PSUM") as mm, \
             tc.tile_pool(name="po", bufs=1, space="PSUM") as po, \
             tc.tile_pool(name="esc", bufs=6) as esc, \
             tc.tile_pool(name="dram", bufs=1, space="DRAM") as dram:

            identity = consts.tile([128, 128], BF16)
            make_identity(nc, identity)
            ones = consts.tile([128, 1], F32)
            nc.vector.memset(ones, 1.0)

            # ---- load the packed blob ----
            braw = big.tile([128, NS, PC], BF16)
            nc.sync.dma_start(
                out=braw, in_=blob_d.rearrange("(n p) c -> p n c", p=128))
            praw = braw[:, :, 0:D]          # [128, 8, 64] bf16 (strided)
            pneg = braw[:, :, D:PC]         # [128, 8, 9]  bf16 (strided)

            # ---- normalize local rows ----
            sq = big.tile([128, NS, D], F32)
            nc.vector.tensor_mul(sq, praw, praw)
            n2 = big.tile([128, NS], F32)
            nc.vector.tensor_reduce(n2, sq, axis=mybir.AxisListType.X,
                                    op=ALU.add)
            inv = _emit_rsqrt(nc, big, n2, NS)
            pnb = big.tile([128, NS, D], BF16)
            for n in range(NS):
                nc.vector.tensor_scalar_mul(pnb[:, n, :], praw[:, n, :],
                                            inv[:, n:n + 1])

            # ---- transpose local pn -> psT [64, 1024], kick off AllGather ----
            psT = big.tile([64, R], BF16)
            for q4 in range(2):
                tpp = tp.tile([64, 512], BF16, tag="tp")
                for q in range(4):
                    nn = 4 * q4 + q
                    nc.tensor.transpose(tpp[:, q * 128:(q + 1) * 128],
                                        pnb[:, nn, :], identity)
                nc.vector.tensor_copy(psT[:, q4 * 512:(q4 + 1) * 512], tpp)

            inb = dram.tile([64, R], BF16)
            outb = dram.tile([NCORES, 64, R], BF16)
            nc.gpsimd.dma_start(inb[:], psT[:])
            nc.gpsimd.collective_compute(
                "AllGather", ALU.bypass,
                replica_groups=[list(range(NCORES))],
                ins=[inb.opt()], outs=[outb.opt()],
            )
            pnT = big.tile([64, NCORES, R], BF16)
            nc.sync.dma_start(out=pnT, in_=outb.rearrange("k d c -> d k c"))
            pnTf = pnT.rearrange("d k c -> d (k c)")      # [64, 8192]

            # ---- probs negatives + pos/diag (overlaps the AllGather) ----
            pnegc = big.tile([128, NS, NNEG], F32)
            nc.vector.tensor_copy(pnegc, pneg)
            eprobs = big.tile([128, NS, NNEG], F32)
            nc.scalar.activation(eprobs.rearrange("p n c -> p (n c)"),
                                 pnegc.rearrange("p n c -> p (n c)"), AF.Exp)
            ps2 = big.tile([128, NS], F32)
            nc.vector.tensor_reduce(ps2, eprobs, axis=mybir.AxisListType.X,
                                    op=ALU.add)

            dq = work.tile([128, NS, D], F32, tag="rowdot")
            nc.vector.tensor_mul(dq, pnb, pnb)
            diag_raw = big.tile([128, NS], F32)
            nc.vector.tensor_reduce(diag_raw, dq, axis=mybir.AxisListType.X,
                                    op=ALU.add)
            # positive partner of chunk n is chunk (n+4)%8, same partition
            ph = work.tile([128, 4, D], F32, tag="rowdot")
            nc.vector.tensor_mul(ph, pnb[:, 0:4, :], pnb[:, 4:8, :])
            pos_raw = big.tile([128, NS], F32)
            nc.vector.tensor_reduce(pos_raw[:, 0:4], ph,
                                    axis=mybir.AxisListType.X, op=ALU.add)
            nc.vector.tensor_copy(pos_raw[:, 4:8], pos_raw[:, 0:4])

            ediag = big.tile([128, NS], F32)
            nc.scalar.activation(ediag, diag_raw, AF.Exp, scale=INV_TEMP)
            epos = big.tile([128, NS], F32)
            nc.scalar.activation(epos, pos_raw, AF.Exp, scale=INV_TEMP)
            pos2 = big.tile([128, NS], F32)
            nc.vector.tensor_scalar_mul(pos2, pos_raw, INV_TEMP)

            # ---- main loop: exp(sim slab) row sums ----
            scols = big.tile([128, NS * JJ], F32)
            for jj in range(JJ):
                c0 = jj * 1024
                for n in range(NS):
                    idx = n * JJ + jj
                    pst = mm.tile([128, 1024], F32, tag="mm")
                    lhsT = psT[:, n * 128:(n + 1) * 128]
                    nc.tensor.matmul(pst[:, 0:512], lhsT,
                                     pnTf[:, c0:c0 + 512],
                                     start=True, stop=True)
                    nc.tensor.matmul(pst[:, 512:1024], lhsT,
                                     pnTf[:, c0 + 512:c0 + 1024],
                                     start=True, stop=True)
                    et = esc.tile([128, 1024], F32, tag="esc")
                    if (idx * N_ACC) % (NS * JJ) < N_ACC:
                        nc.scalar.activation(
                            et, pst, AF.Exp, scale=INV_TEMP,
                            accum_out=scols[:, idx:idx + 1])
                    else:
                        nc.scalar.activation(et, pst, AF.Exp,
                                             scale=INV_TEMP)
                        nc.vector.tensor_reduce(
                            scols[:, idx:idx + 1], et,
                            axis=mybir.AxisListType.X, op=ALU.add)

            # ---- loss tails ----
            stot = big.tile([128, NS], F32)
            nc.vector.tensor_reduce(
                stot, scols.rearrange("p (n j) -> p n j", j=JJ),
                axis=mybir.AxisListType.X, op=ALU.add)
            s1 = big.tile([128, NS], F32)
            nc.vector.tensor_sub(s1, stot, ediag)
            lse1 = big.tile([128, NS], F32)
            nc.scalar.activation(lse1, s1, AF.Ln)
            c1 = big.tile([128, NS], F32)
            nc.vector.tensor_sub(c1, lse1, pos2)
            v12 = big.tile([128, 2], F32)
            nc.vector.tensor_reduce(v12[:, 0:1], c1,
                                    axis=mybir.AxisListType.X, op=ALU.add)

            s2 = big.tile([128, NS], F32)
            nc.vector.tensor_add(s2, ps2, epos)
            # false data-dep on stot so the scheduler cannot hoist the Ln
            # into the exp stream (each hoist costs 2 ACT table swaps)
            nc.vector.scalar_tensor_tensor(
                out=s2, in0=stot, scalar=0.0, in1=s2,
                op0=ALU.mult, op1=ALU.add)
            lse2 = big.tile([128, NS], F32)
            nc.scalar.activation(lse2, s2, AF.Ln)
            c2 = big.tile([128, NS], F32)
            nc.vector.tensor_sub(c2, lse2, pos2)
            nc.vector.tensor_reduce(v12[:, 1:2], c2,
                                    axis=mybir.AxisListType.X, op=ALU.add)

            # ---- partition-sum via ones-matmul, then DMA out ----
            pso = po.tile([1, 2], F32)
            nc.tensor.matmul(pso, ones, v12, start=True, stop=True)
            outsb = big.tile([1, 2], F32)
            nc.vector.tensor_copy(outsb, pso)
            nc.sync.dma_start(out=out_d, in_=outsb)

    nc.compile()
    return nc


_NC_CACHE = None


def _get_nc():
    global _NC_CACHE
    if _NC_CACHE is None:
        _NC_CACHE = build_program()
    return _NC_CACHE


def make_blob(z_i, z_j, probs, target):
    """[8, 1024, 73] bf16: per shard k rows [z_i[k*512:(k+1)*512];
    z_j[...]], cols 0:64 = p row, 64:73 = probs with own class dropped."""
    z_i = np.asarray(z_i, np.float32)
    z_j = np.asarray(z_j, np.float32)
    probs = np.asarray(probs, np.float32)
    t2 = np.concatenate([np.asarray(target), np.asarray(target)])
    keep = np.arange(NCLS)[None, :] != t2[:, None]
    pn9 = probs[keep].reshape(M, NNEG)
    blob = np.empty((NCORES, R, PC), ml_dtypes.bfloat16)
    half = R // 2
    blob[:, :half, :D] = z_i.reshape(NCORES, half, D)
    blob[:, half:, :D] = z_j.reshape(NCORES, half, D)
    blob[:, :half, D:] = pn9[:N].reshape(NCORES, half, NNEG)
    blob[:, half:, D:] = pn9[N:].reshape(NCORES, half, NNEG)
    return blob


def make_in_maps(z_i, z_j, probs, target):
    blob = make_blob(z_i, z_j, probs, target)
    return [{"blob": blob[k]} for k in range(NCORES)]


def _assemble(results):
    parts = np.stack([results[k]["out"].reshape(2) for k in range(NCORES)])
    total = parts.sum(axis=0) / np.float32(M)
    return (np.asarray(np.float32(total[0])), np.asarray(np.float32(total[1])))


class _CachedRunner:
    """run_bass_via_pjrt with the jitted executable built once and the
    donated output zeros created on device (each host->device array costs
    ~70ms of tunnel latency, so per-call traffic is 1 input array)."""

    def __init__(self, nc, n_cores):
        import jax
        import jax.numpy as jnp
        from jax.sharding import Mesh, PartitionSpec, NamedSharding
        import warnings
        with warnings.catch_warnings():
            warnings.simplefilter("ignore")
            from jax.experimental.shard_map import shard_map
        from concourse import bass2jax

        bass2jax.install_neuronx_cc_hook()
        self._jax = jax
        self._np = np
        partition_name = (nc.partition_id_tensor.name
                          if nc.partition_id_tensor else None)

        in_names, out_names, out_avals, zero_shapes = [], [], [], []
        for alloc in nc.m.functions[0].allocations:
            if not isinstance(alloc, mybir.MemoryLocationSet):
                continue
            name = alloc.memorylocations[0].name
            if alloc.kind == "ExternalInput":
                if name != partition_name:
                    in_names.append(name)
            elif alloc.kind == "ExternalOutput":
                out_names.append(name)
                shape = tuple(alloc.tensor_shape)
                dtype = mybir.dt.np(alloc.dtype)
                out_avals.append(jax.core.ShapedArray(shape, dtype))
                zero_shapes.append((shape, dtype))
        n_params = len(in_names)
        n_outs = len(out_avals)
        all_in_names = list(in_names) + list(out_names)
        if partition_name is not None:
            all_in_names.append(partition_name)
        donate = tuple(range(n_params, n_params + n_outs))
        self._in_names = in_names
        self._out_names = out_names
        self._out_avals = out_avals
        self._n_cores = n_cores

        def _body(*args):
            operands = list(args)
            if partition_name is not None:
                operands.append(bass2jax.partition_id_tensor())
            outs = bass2jax._bass_exec_p.bind(
                *operands,
                out_avals=tuple(out_avals),
                in_names=tuple(all_in_names),
                out_names=tuple(out_names),
                lowering_input_output_aliases=(),
                sim_require_finite=True,
                sim_require_nnan=True,
                nc=nc,
            )
            return tuple(outs)

        devices = jax.devices()[:n_cores]
        mesh = Mesh(np.asarray(devices), ("core",))
        in_specs = (PartitionSpec("core"),) * (n_params + n_outs)
        out_specs = (PartitionSpec("core"),) * len(out_names)
        self._sharded = jax.jit(
            shard_map(_body, mesh=mesh, in_specs=in_specs,
                      out_specs=out_specs, check_rep=False),
            donate_argnums=donate, keep_unused=True,
        )
        csh = NamedSharding(mesh, PartitionSpec("core"))

        def _zeros():
            return tuple(
                jnp.zeros((n_cores * s[0], *s[1:]), d)
                for s, d in zero_shapes)

        self._zf = jax.jit(_zeros, out_shardings=(csh,) * n_outs)

    def run(self, in_maps):
        np_ = self._np
        per_core = [[np_.asarray(m[name]) for name in self._in_names]
                    for m in in_maps]
        concat_in = [
            np_.concatenate([per_core[c][i] for c in range(self._n_cores)],
                            axis=0)
            for i in range(len(self._in_names))
        ]
        zeros = self._zf()
        out_arrs = self._sharded(*concat_in, *zeros)
        return [
            {
                name: np_.asarray(out_arrs[i]).reshape(
                    self._n_cores, *self._out_avals[i].shape)[c]
                for i, name in enumerate(self._out_names)
            }
            for c in range(self._n_cores)
        ]


_RUNNER = None
_RUNNER_FAILED = False


def kernel(z_i, z_j, probs, target, neg_idx):
    # neg_idx is the fixed structured NT-Xent mask (all columns except self
    # and positive); its effect is computed analytically, so it's never read.
    del neg_idx
    global _RUNNER, _RUNNER_FAILED
    nc = _get_nc()
    in_maps = make_in_maps(z_i, z_j, probs, target)

    if _RUNNER is not None:
        try:
            return _assemble(_RUNNER.run(in_maps))
        except Exception:
            # tunnel hiccup or runner breakage: permanently fall back to
            # the stock dispatch path
            _RUNNER = None
            _RUNNER_FAILED = True

    res = run_bass_kernel_spmd(nc, in_maps, list(range(NCORES)))
    out = _assemble(res.results)

    if not _RUNNER_FAILED:
        try:
            runner = _CachedRunner(nc, NCORES)
            chk = _assemble(runner.run(in_maps))
            if (abs(float(chk[0]) - float(out[0])) <= 1e-4 * abs(float(out[0]))
                    and abs(float(chk[1]) - float(out[1]))
                    <= 1e-4 * abs(float(out[1]))):
                _RUNNER = runner
            else:
                _RUNNER_FAILED = True
        except Exception:
            _RUNNER_FAILED = True
    return out


# revision 5
# speedup vs baseline: 9.3487x; 1.0155x over previous
"""NT-Xent loss kernel for Trainium2, 8-core SPMD with on-device AllGather.

Math: with p = cat(z_i, z_j) [8192, 64], pn = p / max(||p||, 1e-8),
sim = 2 * pn @ pn.T (TEMP=0.5), the reference's gather-based losses reduce to
  loss1 = mean_r( log(sum_c exp(sim[r,c]) - exp(sim[r,r])) - pos_r )
  loss2 = mean_r( log(exp(pos_r) + sum_{c != t_r} exp(probs[r,c])) - pos_r )
where pos_r = sim[r, partner(r)].  sim entries lie in [-2, 2] so no max-shift
is needed.  The huge neg_idx input is a fixed structured mask and is never
read; the probs negative selection (drop own class) is 8192x10 index prep
done on host.

Both losses are sums of per-row terms, and each row term depends only on the
full column set -- so any symmetric permutation of the row order leaves them
unchanged.  We permute rows so that each core's 1024 rows are
[z_i[k*512:(k+1)*512]; z_j[k*512:(k+1)*512]]: every row's positive partner
lives on the same core at a fixed local offset (+-512 rows = chunk n <-> n+4),
which keeps the SPMD program branch-free.

Each core receives ONLY its own 1024 rows packed with its probs negatives in
one bf16 blob [1024, 73] (the wall-clock cost of a call is dominated by
per-array host->device transfer latency over the axon tunnel, so everything
rides in a single small array).  On device: normalize locally, transpose,
AllGather the transposed normalized rows (bf16, 128KB/core) across the 8
cores over the on-chip links, then compute the [1024 x 8192] slab of
exp(sim) and both loss tails.  Host sums the 8 partial pairs.

Dispatch: bass_utils.run_bass_kernel_spmd rebuilds its jax.jit on every call
(full retrace + relower + NEFF rewrap + executable reload over the tunnel,
~200ms) and ships per-core arrays individually (~70ms each).  The first
kernel() call runs through run_bass_kernel_spmd; it also builds a
semantically identical runner around the same _bass_exec_p primitive with
the jitted executable cached, verifies it against the stock result, and
subsequent calls dispatch through that.
"""

import numpy as np
import ml_dtypes

import concourse.bass as bass
import concourse.bacc as bacc
import concourse.tile as tile
from concourse import mybir
from concourse.masks import make_identity
from concourse.bass_utils import run_bass_kernel_spmd

N = 4096
D = 64
M = 2 * N            # 8192 rows of sim
NCORES = 8
R = M // NCORES      # 1024 rows per core
NS = R // 128        # 8 row-chunks of 128 per core
JJ = 8               # column groups of 1024 in the main loop
NCLS = 10
NNEG = NCLS - 1      # 9 probs negatives per row
PC = D + NNEG        # 73 blob columns
INV_TEMP = 2.0       # 1 / 0.5
F32 = mybir.dt.float32
BF16 = mybir.dt.bfloat16
NEWTON_ITERS = 2
N_ACC = 28           # of the 64 row-sum reductions, how many use ACT accum_out

AF = mybir.ActivationFunctionType
ALU = mybir.AluOpType


def _emit_rsqrt(nc, pool, n2, nchunk):
    """inv = 1/max(sqrt(n2), 1e-8) on DVE: quake-style magic-constant seed +
    Newton steps (keeps ACT exclusively on Exp/Ln so its table never
    thrashes)."""
    eng = nc.vector
    I32 = mybir.dt.int32
    inv = pool.tile([128, nchunk], F32, tag="rs_inv")
    eng.tensor_scalar(inv.bitcast(I32), n2.bitcast(I32), 1, None,
                      ALU.arith_shift_right)
    eng.tensor_scalar(inv.bitcast(I32), inv.bitcast(I32), -1, 0x5F3759DF,
                      ALU.mult, ALU.add)
    t2 = pool.tile([128, nchunk], F32, tag="rs_t2")
    for _ in range(NEWTON_ITERS):
        eng.tensor_mul(t2, inv, inv)
        eng.tensor_mul(t2, t2, n2)
        eng.tensor_scalar(t2, t2, -0.5, 1.5, ALU.mult, ALU.add)
        eng.tensor_mul(inv, inv, t2)
    eng.tensor_scalar_min(inv, inv, 1e8)
    return inv


def build_program():
    nc = bacc.Bacc("TRN2", target_bir_lowering=False, debug=False,
                   num_devices=NCORES)

    blob_d = nc.dram_tensor("blob", [R, PC], BF16, kind="ExternalInput").ap()
    out_d = nc.dram_tensor("out", [1, 2], F32, kind="ExternalOutput").ap()

    with tile.TileContext(nc) as tc:
        with tc.tile_pool(name="consts", bufs=1) as consts, \
             tc.tile_pool(name="big", bufs=1) as big, \
             tc.tile_pool(name="work", bufs=2) as work, \
             tc.tile_pool(name="tp", bufs=2, space="PSUM") as tp, \
             tc.tile_pool(name="mm", bufs=2, space="PSUM") as mm, \
             tc.tile_pool(name="po", bufs=1, space="PSUM") as po, \
             tc.tile_pool(name="esc", bufs=6) as esc, \
             tc.tile_pool(name="dram", bufs=1, space="DRAM") as dram:

            identity = consts.tile([128, 128], BF16)
            make_identity(nc, identity)
            ones = consts.tile([128, 1], F32)
            nc.vector.memset(ones, 1.0)

            # ---- load the packed blob ----
            braw = big.tile([128, NS, PC], BF16)
            nc.sync.dma_start(
                out=braw, in_=blob_d.rearrange("(n p) c -> p n c", p=128))
            praw = braw[:, :, 0:D]          # [128, 8, 64] bf16 (strided)
            pneg = braw[:, :, D:PC]         # [128, 8, 9]  bf16 (strided)

            # ---- normalize local rows ----
            sq = big.tile([128, NS, D], F32)
            nc.vector.tensor_mul(sq, praw, praw)
            n2 = big.tile([128, NS], F32)
            nc.vector.tensor_reduce(n2, sq, axis=mybir.AxisListType.X,
                                    op=ALU.add)
            inv = _emit_rsqrt(nc, big, n2, NS)
            pnb = big.tile([128, NS, D], BF16)
            for n in range(NS):
                nc.vector.tensor_scalar_mul(pnb[:, n, :], praw[:, n, :],
                                            inv[:, n:n + 1])

            # ---- transpose local pn -> psT [64, 1024], kick off AllGather ----
            psT = big.tile([64, R], BF16)
            for q4 in range(2):
                tpp = tp.tile([64, 512], BF16, tag="tp")
                for q in range(4):
                    nn = 4 * q4 + q
                    nc.tensor.transpose(tpp[:, q * 128:(q + 1) * 128],
                                        pnb[:, nn, :], identity)
                nc.vector.tensor_copy(psT[:, q4 * 512:(q4 + 1) * 512], tpp)

            inb = dram.tile([64, R], BF16)
            outb = dram.tile([NCORES, 64, R], BF16)
            nc.gpsimd.dma_start(inb[:], psT[:])
            nc.gpsimd.collective_compute(
                "AllGather", ALU.bypass,
                replica_groups=[list(range(NCORES))],
                ins=[inb.opt()], outs=[outb.opt()],
            )
            pnT = big.tile([64, NCORES, R], BF16)
            nc.sync.dma_start(out=pnT, in_=outb.rearrange("k d c -> d k c"))
            pnTf = pnT.rearrange("d k c -> d (k c)")      # [64, 8192]

            # ---- probs negatives + pos/diag (overlaps the AllGather) ----
            pnegc = big.tile([128, NS, NNEG], F32)
            nc.vector.tensor_copy(pnegc, pneg)
            eprobs = big.tile([128, NS, NNEG], F32)
            nc.scalar.activation(eprobs.rearrange("p n c -> p (n c)"),
                                 pnegc.rearrange("p n c -> p (n c)"), AF.Exp)
            ps2 = big.tile([128, NS], F32)
            nc.vector.tensor_reduce(ps2, eprobs, axis=mybir.AxisListType.X,
                                    op=ALU.add)

            dq = work.tile([128, NS, D], F32, tag="rowdot")
            nc.vector.tensor_mul(dq, pnb, pnb)
            diag_raw = big.tile([128, NS], F32)
            nc.vector.tensor_reduce(diag_raw, dq, axis=mybir.AxisListType.X,
                                    op=ALU.add)
            # positive partner of chunk n is chunk (n+4)%8, same partition
            ph = work.tile([128, 4, D], F32, tag="rowdot")
            nc.vector.tensor_mul(ph, pnb[:, 0:4, :], pnb[:, 4:8, :])
            pos_raw = big.tile([128, NS], F32)
            nc.vector.tensor_reduce(pos_raw[:, 0:4], ph,
                                    axis=mybir.AxisListType.X, op=ALU.add)
            nc.vector.tensor_copy(pos_raw[:, 4:8], pos_raw[:, 0:4])

            ediag = big.tile([128, NS], F32)
            nc.scalar.activation(ediag, diag_raw, AF.Exp, scale=INV_TEMP)
            epos = big.tile([128, NS], F32)
            nc.scalar.activation(epos, pos_raw, AF.Exp, scale=INV_TEMP)
            pos2 = big.tile([128, NS], F32)
            nc.vector.tensor_scalar_mul(pos2, pos_raw, INV_TEMP)

            # ---- main loop: exp(sim slab) row sums ----
            scols = big.tile([128, NS * JJ], F32)
            for jj in range(JJ):
                c0 = jj * 1024
                for n in range(NS):
                    idx = n * JJ + jj
                    pst = mm.tile([128, 1024], F32, tag="mm")
                    lhsT = psT[:, n * 128:(n + 1) * 128]
                    nc.tensor.matmul(pst[:, 0:512], lhsT,
                                     pnTf[:, c0:c0 + 512],
                                     start=True, stop=True)
                    nc.tensor.matmul(pst[:, 512:1024], lhsT,
                                     pnTf[:, c0 + 512:c0 + 1024],
                                     start=True, stop=True)
                    et = esc.tile([128, 1024], F32, tag="esc")
                    if (idx * N_ACC) % (NS * JJ) < N_ACC:
                        nc.scalar.activation(
                            et, pst, AF.Exp, scale=INV_TEMP,
                            accum_out=scols[:, idx:idx + 1])
                    else:
                        nc.scalar.activation(et, pst, AF.Exp,
                                             scale=INV_TEMP)
                        nc.vector.tensor_reduce(
                            scols[:, idx:idx + 1], et,
                            axis=mybir.AxisListType.X, op=ALU.add)

            # ---- loss tails ----
            stot = big.tile([128, NS], F32)
            nc.vector.tensor_reduce(
                stot, scols.rearrange("p (n j) -> p n j", j=JJ),
                axis=mybir.AxisListType.X, op=ALU.add)
            s1 = big.tile([128, NS], F32)
            nc.vector.tensor_sub(s1, stot, ediag)
            lse1 = big.tile([128, NS], F32)
            nc.scalar.activation(lse1, s1, AF.Ln)
            c1 = big.tile([128, NS], F32)
            nc.vector.tensor_sub(c1, lse1, pos2)
            v12 = big.tile([128, 2], F32)
            nc.vector.tensor_reduce(v12[:, 0:1], c1,
                                    axis=mybir.AxisListType.X, op=ALU.add)

            s2 = big.tile([128, NS], F32)
            nc.vector.tensor_add(s2, ps2, epos)
            # false data-dep on stot so the scheduler cannot hoist the Ln
            # into the exp stream (each hoist costs 2 ACT table swaps)
            nc.vector.scalar_tensor_tensor(
                out=s2, in0=stot, scalar=0.0, in1=s2,
                op0=ALU.mult, op1=ALU.add)
            lse2 = big.tile([128, NS], F32)
            nc.scalar.activation(lse2, s2, AF.Ln)
            c2 = big.tile([128, NS], F32)
            nc.vector.tensor_sub(c2, lse2, pos2)
            nc.vector.tensor_reduce(v12[:, 1:2], c2,
                                    axis=mybir.AxisListType.X, op=ALU.add)

            # ---- partition-sum via ones-matmul, then DMA out ----
            pso = po.tile([1, 2], F32)
            nc.tensor.matmul(pso, ones, v12, start=True, stop=True)
            outsb = big.tile([1, 2], F32)
            nc.vector.tensor_copy(outsb, pso)
            nc.sync.dma_start(out=out_d, in_=outsb)

    nc.compile()
    return nc


_NC_CACHE = None


def _get_nc():
    global _NC_CACHE
    if _NC_CACHE is None:
        _NC_CACHE = build_program()
    return _NC_CACHE


def make_blob(z_i, z_j, probs, target):
    """[8, 1024, 73] bf16: per shard k rows [z_i[k*512:(k+1)*512];
    z_j[...]], cols 0:64 = p row, 64:73 = probs with own class dropped."""
    z_i = np.asarray(z_i, np.float32)
    z_j = np.asarray(z_j, np.float32)
    probs = np.asarray(probs, np.float32)
    t2 = np.concatenate([np.asarray(target), np.asarray(target)])
    keep = np.arange(NCLS)[None, :] != t2[:, None]
    pn9 = probs[keep].reshape(M, NNEG)
    blob = np.empty((NCORES, R, PC), ml_dtypes.bfloat16)
    half = R // 2
    blob[:, :half, :D] = z_i.reshape(NCORES, half, D)
    blob[:, half:, :D] = z_j.reshape(NCORES, half, D)
    blob[:, :half, D:] = pn9[:N].reshape(NCORES, half, NNEG)
    blob[:, half:, D:] = pn9[N:].reshape(NCORES, half, NNEG)
    return blob


def make_in_maps(z_i, z_j, probs, target):
    blob = make_blob(z_i, z_j, probs, target)
    return [{"blob": blob[k]} for k in range(NCORES)]


def _assemble(results):
    parts = np.stack([results[k]["out"].reshape(2) for k in range(NCORES)])
    total = parts.sum(axis=0) / np.float32(M)
    return (np.asarray(np.float32(total[0])), np.asarray(np.float32(total[1])))


class _CachedRunner:
    """run_bass_via_pjrt with the jitted executable built once and the
    donated output zeros created on device (each host->device array costs
    ~70ms of tunnel latency, so per-call traffic is 1 input array)."""

    def __init__(self, nc, n_cores):
        import jax
        import jax.numpy as jnp
        from jax.sharding import Mesh, PartitionSpec, NamedSharding
        import warnings
        with warnings.catch_warnings():
            warnings.simplefilter("ignore")
            from jax.experimental.shard_map import shard_map
        from concourse import bass2jax

        bass2jax.install_neuronx_cc_hook()
        self._jax = jax
        self._np = np
        partition_name = (nc.partition_id_tensor.name
                          if nc.partition_id_tensor else None)

        in_names, out_names, out_avals, zero_shapes = [], [], [], []
        for alloc in nc.m.functions[0].allocations:
            if not isinstance(alloc, mybir.MemoryLocationSet):
                continue
            name = alloc.memorylocations[0].name
            if alloc.kind == "ExternalInput":
                if name != partition_name:
                    in_names.append(name)
            elif alloc.kind == "ExternalOutput":
                out_names.append(name)
                shape = tuple(alloc.tensor_shape)
                dtype = mybir.dt.np(alloc.dtype)
                out_avals.append(jax.core.ShapedArray(shape, dtype))
                zero_shapes.append((shape, dtype))
        n_params = len(in_names)
        n_outs = len(out_avals)
        all_in_names = list(in_names) + list(out_names)
        if partition_name is not None:
            all_in_names.append(partition_name)
        donate = tuple(range(n_params, n_params + n_outs))
        self._in_names = in_names
        self._out_names = out_names
        self._out_avals = out_avals
        self._n_cores = n_cores

        def _body(*args):
            operands = list(args)
            if partition_name is not None:
                operands.append(bass2jax.partition_id_tensor())
            outs = bass2jax._bass_exec_p.bind(
                *operands,
                out_avals=tuple(out_avals),
                in_names=tuple(all_in_names),
                out_names=tuple(out_names),
                lowering_input_output_aliases=(),
                sim_require_finite=True,
                sim_require_nnan=True,
                nc=nc,
            )
            return tuple(outs)

        devices = jax.devices()[:n_cores]
        mesh = Mesh(np.asarray(devices), ("core",))
        in_specs = (PartitionSpec("core"),) * (n_params + n_outs)
        out_specs = (PartitionSpec("core"),) * len(out_names)
        self._sharded = jax.jit(
            shard_map(_body, mesh=mesh, in_specs=in_specs,
                      out_specs=out_specs, check_rep=False),
            donate_argnums=donate, keep_unused=True,
        )
        csh = NamedSharding(mesh, PartitionSpec("core"))

        def _zeros():
            return tuple(
                jnp.zeros((n_cores * s[0], *s[1:]), d)
                for s, d in zero_shapes)

        self._zf = jax.jit(_zeros, out_shardings=(csh,) * n_outs)

    def run(self, in_maps):
        np_ = self._np
        per_core = [[np_.asarray(m[name]) for name in self._in_names]
                    for m in in_maps]
        concat_in = [
            np_.concatenate([per_core[c][i] for c in range(self._n_cores)],
                            axis=0)
            for i in range(len(self._in_names))
        ]
        zeros = self._zf()
        out_arrs = self._sharded(*concat_in, *zeros)
        return [
            {
                name: np_.asarray(out_arrs[i]).reshape(
                    self._n_cores, *self._out_avals[i].shape)[c]
                for i, name in enumerate(self._out_names)
            }
            for c in range(self._n_cores)
        ]


_RUNNER = None
_RUNNER_FAILED = False


def kernel(z_i, z_j, probs, target, neg_idx):
    # neg_idx is the fixed structured NT-Xent mask (all columns except self
    # and positive); its effect is computed analytically, so it's never read.
    del neg_idx
    global _RUNNER, _RUNNER_FAILED
    nc = _get_nc()
    in_maps = make_in_maps(z_i, z_j, probs, target)

    if _RUNNER is not None:
        try:
            return _assemble(_RUNNER.run(in_maps))
        except Exception:
            # tunnel hiccup or runner breakage: permanently fall back to
            # the stock dispatch path
            _RUNNER = None
            _RUNNER_FAILED = True

    try:
        res = run_bass_kernel_spmd(nc, in_maps, list(range(NCORES)))
        out = _assemble(res.results)
    except Exception:
        # e.g. BASS_TRACE=1 in an env without antenv.axon_hooks
        out = None

    if not _RUNNER_FAILED:
        try:
            runner = _CachedRunner(nc, NCORES)
            chk = _assemble(runner.run(in_maps))
            if out is None:
                out = chk
                _RUNNER = runner
            elif (abs(float(chk[0]) - float(out[0]))
                    <= 1e-4 * abs(float(out[0]))
                    and abs(float(chk[1]) - float(out[1]))
                    <= 1e-4 * abs(float(out[1]))):
                _RUNNER = runner
            else:
                _RUNNER_FAILED = True
        except Exception:
            _RUNNER_FAILED = True
    if out is None:
        raise RuntimeError(
            "both the stock run_bass_kernel_spmd dispatch and the cached "
            "runner failed")
    return out
